# revision 21
# baseline (speedup 1.0000x reference)
"""GAT-style masked-softmax attention kernel for Trainium2 (8 NeuronCores).

Problem (per batch b of 32):
    e   = leaky_relu(h @ a1 + (g @ a2)^T, 0.2)        # (N, M)
    att = softmax(where(adj > 0, e, -9e15), axis=-1)  # (N, M)
    out = (att * adj.sum(-1, keepdims=True)) @ g      # (N, D)

Strategy (pure data parallel over B=32 -> 4 batches/core):
  * Mask folded into the scores: softmax(where(a>0, e, -inf)) ==
    softmax(leaky_relu(u_i + v_j + BETA*a_ij - BETA)) for large BETA,
    because Prelu of a hugely negative number stays hugely negative and
    exp() of it underflows to ~0 relative to real scores.
  * adj is cast int32->bf16 during the HBM load (SWDGE cast-on-load),
    landing in natural [i, j] layout. No on-chip transpose of the mask:
    BETA*a^T is produced directly in PSUM by TensorE matmuls of af
    blocks against a BETA*I identity (transpose-by-matmul), accumulated
    with the broadcast of u (K=1 ones matmuls, exact bf16 hi+lo split).
  * ACT then does Prelu(B + (v_j - BETA)) and Exp -> s^T bf16; s^T feeds
    the output matmul (weight-stationary g) and a ones-matmul rowsum.
  * deg_i = sum_j a_ij rides a DVE tensor_scalar accumulate over af
    (natural layout, free-axis reduction); deg and u bounce through DRAM
    once per batch to become row vectors (bf16 hi/lo pairs).
  * Epilogue scales out^T by deg/rowsum row-wise; output stored bf16
    transposed, host un-transposes and casts to f32.

Self-contained: hardcodes shapes B,N,M,D = 32,1024,1024,128 on 8 cores.
"""

import sys

if "/opt/trn_rl_repo" not in sys.path:
    sys.path.insert(0, "/opt/trn_rl_repo")

import numpy as np
import ml_dtypes

import concourse.bacc as bacc
import concourse.mybir as mybir
import concourse.tile as tile
import concourse.bass_utils as bass_utils

F32 = mybir.dt.float32
F32R = mybir.dt.float32r
BF16 = mybir.dt.bfloat16
I32 = mybir.dt.int32
OP = mybir.AluOpType
AF = mybir.ActivationFunctionType

B, N, M, D = 32, 1024, 1024, 128
NCORES = 8
BPC = B // NCORES  # batches per core
NI = N // 128      # i blocks
NJ = M // 128      # j blocks
BETA = 100.0


def build_bass():
    nc = bacc.Bacc("TRN2", target_bir_lowering=False, debug=False)

    h_in = nc.dram_tensor("input1", [BPC, N, D], F32, kind="ExternalInput").ap()
    g_in = nc.dram_tensor("input2", [BPC, M, D], F32, kind="ExternalInput").ap()
    adj_in = nc.dram_tensor("adj", [BPC, N, M], I32, kind="ExternalInput").ap()
    a1_in = nc.dram_tensor("a1", [D, 1], F32, kind="ExternalInput").ap()
    a2_in = nc.dram_tensor("a2", [D, 1], F32, kind="ExternalInput").ap()
    beye_in = nc.dram_tensor("beye", [128, 128], BF16, kind="ExternalInput").ap()
    eyef_in = nc.dram_tensor("eyef", [128, 128], F32, kind="ExternalInput").ap()
    # out^T bf16: host transposes (0,2,1) + casts f32 after gather
    out_d = nc.dram_tensor("out", [BPC, D, N], BF16, kind="ExternalOutput").ap()


    with tile.TileContext(nc) as tc:
        with (
            tc.tile_pool(name="singles", bufs=1) as singles,
            tc.tile_pool(name="hg", bufs=3) as hg_pool,
            tc.tile_pool(name="gbf", bufs=3) as gbf_pool,
            tc.tile_pool(name="afp", bufs=2) as af_pool,
            tc.tile_pool(name="plp", bufs=2) as pl_pool,
            tc.tile_pool(name="st", bufs=2) as st_pool,
            tc.tile_pool(name="small", bufs=4) as small,
            tc.tile_pool(name="folds", bufs=2) as folds,
            tc.tile_pool(name="rows", bufs=3) as rows_pool,
            tc.tile_pool(name="rows1", bufs=1) as rows1_pool,
            tc.tile_pool(name="psB", bufs=4, space="PSUM") as psB,
            tc.tile_pool(name="psO", bufs=1, space="PSUM") as psO,
            tc.tile_pool(name="psR", bufs=1, space="PSUM") as psR,
        ):
            # ---- static prep ------------------------------------------------
            ones_row = singles.tile([1, 128], BF16)
            nc.vector.memset(ones_row[:], 1.0)
            ones_sq_bf = singles.tile([128, 128], BF16)
            nc.vector.memset(ones_sq_bf[:], 1.0)
            beye = singles.tile([128, 128], BF16)
            nc.sync.dma_start(beye[:], beye_in)
            eyef = singles.tile([128, 128], F32)
            nc.sync.dma_start(eyef[:], eyef_in)

            a1row = singles.tile([1, D], F32)
            nc.gpsimd.dma_start(a1row[:], a1_in.transpose((1, 0)))
            a2row = singles.tile([1, D], F32)
            nc.gpsimd.dma_start(a2row[:], a2_in.transpose((1, 0)))
            ones_f = singles.tile([1, 128], F32)
            nc.vector.memset(ones_f[:], 1.0)
            ones_r = singles.tile([1, 128], F32R)
            nc.vector.tensor_copy(ones_r[:], ones_f[:])

            a1bc = singles.tile([128, D], F32)
            a2bc = singles.tile([128, D], F32)
            bc_ps = psB.tile([128, 512], F32, tag="B")
            nc.tensor.matmul(
                bc_ps[:, 0:D], ones_f[:], a1row[:], start=True, stop=False
            )
            nc.tensor.matmul(
                bc_ps[:, D : 2 * D], ones_f[:], a2row[:], start=False, stop=True
            )
            nc.vector.tensor_copy(a1bc[:], bc_ps[:, 0:D])
            nc.vector.tensor_copy(a2bc[:], bc_ps[:, D : 2 * D])


            def prefetch_hg(b):
                h_t = hg_pool.tile([128, NI, D], F32, tag="h")
                nc.sync.dma_start(
                    h_t[:], h_in[b].rearrange("(ib p) d -> p ib d", p=128)
                )
                g_t = hg_pool.tile([128, NJ, D], F32, tag="g")
                nc.sync.dma_start(
                    g_t[:], g_in[b].rearrange("(jb p) d -> p jb d", p=128)
                )
                return h_t, g_t

            def prefetch_af(b):
                # adj int32 -> bf16 cast during the load (SWDGE), 4 adjacent
                # j-chunk DMAs: the first score matmuls gate on 1/4 of it
                af = af_pool.tile([128, 4, NI, 256], BF16)
                for jc in range(4):
                    nc.gpsimd.dma_start(
                        af[:, jc],
                        adj_in[b][:, jc * 256 : (jc + 1) * 256].rearrange(
                            "(ib p) m -> p ib m", p=128
                        ),
                    )
                return af

            def stage_deg_head(b, af):
                # deg_i = sum_j a_ij for the CURRENT batch: 4 SWDGE
                # accumulate-DMAs (512B descriptors) fold j 1024 -> 256
                # (sums <= 4, exact in bf16), then one DVE reduce
                dacc = folds.tile([128, NI, 256], BF16, tag="dacc")
                nc.gpsimd.dma_start(dacc[:], af[:, 0])
                for t in range(1, 4):
                    nc.gpsimd.dma_start(dacc[:], af[:, t], accum_op=OP.add)
                degc = small.tile([128, NI], F32, tag="degc")
                nc.vector.tensor_reduce(
                    degc[:], dacc[:], mybir.AxisListType.X, OP.add
                )
                return degc

            def stage_deg_tail(b, degc):
                # deg columns -> row form, all on-chip: identity matmul
                # transposes [128, NI] -> [NI, 128] in PSUM, DVE rounds to
                # f32r, one 8-descriptor SBUF->SBUF DMA flattens to [1, N]
                degT = psB.tile([NI, 128], F32, tag="B")
                nc.tensor.matmul(degT[:], degc[:], eyef[:], start=True, stop=True)
                degrows8 = small.tile([NI, 128], F32R, tag="degrows8")
                nc.vector.tensor_copy(degrows8[:], degT[:])
                degrow_r = rows_pool.tile([1, N], F32R, tag="degrow_r")
                nc.scalar.dma_start(degrow_r[:], degrows8[:])
                return degrow_r

            def stage_uv_head(b, h_t, g_t):
                # bf16 g for the output matmul
                g_bf = gbf_pool.tile([128, NJ, D], BF16)
                nc.vector.tensor_copy(g_bf[:], g_t[:])
                # u/v projections (u_i = h_i . a1, v_j = g_j . a2)
                ucols = small.tile([128, NI], F32, tag="ucols")
                vcols = small.tile([128, NJ], F32, tag="vcols")
                uscr = small.tile([128, D], F32, tag="uscr")
                for ib in range(NI):
                    nc.vector.scalar_tensor_tensor(
                        uscr[:], h_t[:, ib, :], 0.0, a1bc[:],
                        OP.bypass, OP.mult, accum_out=ucols[:, ib : ib + 1],
                    )
                for jb in range(NJ):
                    nc.vector.scalar_tensor_tensor(
                        uscr[:], g_t[:, jb, :], 0.0, a2bc[:],
                        OP.bypass, OP.mult, accum_out=vcols[:, jb : jb + 1],
                    )
                # bias for the Prelu pass: v_j - BETA (per-partition)
                biasv = small.tile([128, NJ], F32, tag="biasv")
                nc.vector.tensor_scalar(
                    biasv[:], vcols[:], BETA, None, OP.subtract
                )
                return g_bf, biasv, ucols

            def stage_uv_tail(b, ucols):
                # u columns -> row form, all on-chip (same trick as deg)
                uT = psB.tile([NI, 128], F32, tag="B")
                nc.tensor.matmul(uT[:], ucols[:], eyef[:], start=True, stop=True)
                urows8 = small.tile([NI, 128], F32R, tag="urows8")
                nc.vector.tensor_copy(urows8[:], uT[:])
                urow_r = rows_pool.tile([1, N], F32R, tag="urow_r")
                nc.scalar.dma_start(urow_r[:], urows8[:])
                return urow_r

            hg0 = prefetch_hg(0)
            af = prefetch_af(0)
            hg1 = prefetch_hg(1)
            uv0 = stage_uv_head(0, *hg0)
            u0 = stage_uv_tail(0, uv0[2])
            uv1 = stage_uv_head(1, *hg1)
            uvq = [(uv0[0], uv0[1], uv0[2]), (uv1[0], uv1[1], uv1[2])]
            urow_q = [u0]
            for b in range(BPC):
                g_bf, biasv = uvq[0][0], uvq[0][1]
                urow_r = urow_q[0]

                # this batch's deg folds first (af resident), then next
                # af load
                degc = stage_deg_head(b, af)
                if b + 1 < BPC:
                    af_next = prefetch_af(b + 1)
                if b + 2 < BPC:
                    hg = prefetch_hg(b + 2)
                    uvn = stage_uv_head(b + 2, *hg)
                    uvq.append(uvn)
                    urow_q.append(stage_uv_tail(b + 2, uvn[2]))

                outT_ps = psO.tile([128, N], F32, tag="o")
                rs_ps = psR.tile([128, N], F32, tag="r")
                sT = st_pool.tile([128, NJ, N], BF16)
                pl = None
                for jb in range(NJ):
                    # scores^T for block jb: B[j', i] = BETA*a[i, j] + u_i
                    # built per 512-col half (one PSUM bank each)
                    jc, j0 = jb // 2, (jb % 2) * 128
                    if jb % 4 == 0:
                        pl = pl_pool.tile([128, 4, N], F32)
                    for half in range(2):
                        fs = slice(half * 512, (half + 1) * 512)
                        Bh = psB.tile([128, 512], F32, tag="B")
                        for ib4 in range(4):
                            ib = half * 4 + ib4
                            nc.tensor.matmul(
                                Bh[:, ib4 * 128 : (ib4 + 1) * 128],
                                af[:, jc, ib, j0 : j0 + 128],
                                beye[:],
                                start=(ib4 == 0), stop=False,
                            )
                        nc.tensor.matmul(
                            Bh[:], ones_r[:], urow_r[:, fs],
                            start=False, stop=True,
                        )
                        nc.scalar.activation(
                            pl[:, jb % 4, fs], Bh[:], AF.Prelu,
                            bias=biasv[:, jb : jb + 1], alpha=0.2,
                        )
                    if jb % 4 == 3:
                        nc.scalar.activation(
                            sT[:, jb - 3 : jb + 1, :], pl[:], AF.Exp
                        )
                        for j2 in range(jb - 3, jb + 1):
                            for half in range(2):
                                fs = slice(half * 512, (half + 1) * 512)
                                nc.tensor.matmul(
                                    outT_ps[:, fs], g_bf[:, j2, :],
                                    sT[:, j2, fs],
                                    start=(j2 == 0), stop=(j2 == NJ - 1),
                                )
                                nc.tensor.matmul(
                                    rs_ps[:, fs], ones_sq_bf[:],
                                    sT[:, j2, fs],
                                    start=(j2 == 0), stop=(j2 == NJ - 1),
                                )

                # batch 1's u row transpose: emitted after batch 0's
                # matmuls so it doesn't gate the startup tensor stream
                if b == 0 and BPC > 1:
                    urow_q.insert(1, stage_uv_tail(1, uvq[1][2]))
                degrow_r = stage_deg_tail(b, degc)

                # ---- epilogue: out^T * (deg / rowsum), all row-form ---------
                rrow = rows1_pool.tile([128, N], F32, tag="rrow")
                nc.vector.reciprocal_approx_fast(rrow[:], rs_ps[:])
                fac = rows1_pool.tile([128, N], F32, tag="fac")
                for half in range(2):
                    fs = slice(half * 512, (half + 1) * 512)
                    deg_ph = psB.tile([128, 512], F32, tag="B")
                    nc.tensor.matmul(
                        deg_ph[:], ones_r[:], degrow_r[:, fs],
                        start=True, stop=True,
                    )
                    nc.vector.tensor_tensor(
                        fac[:, fs], deg_ph[:], rrow[:, fs], OP.mult
                    )
                outsbT = rows1_pool.tile([128, N], BF16, tag="outsbT")
                nc.vector.tensor_tensor(outsbT[:], outT_ps[:], fac[:], OP.mult)
                nc.sync.dma_start(out_d[b], outsbT[:])
                uvq.pop(0)
                urow_q.pop(0)
                if b + 1 < BPC:
                    af = af_next

    nc.compile()
    return nc


_CACHE = {}


def _get_nc():
    if "nc" not in _CACHE:
        _CACHE["nc"] = build_bass()
    return _CACHE["nc"]


def _make_in_maps(input1, input2, adj, a1, a2):
    input1 = np.ascontiguousarray(np.asarray(input1, dtype=np.float32))
    input2 = np.ascontiguousarray(np.asarray(input2, dtype=np.float32))
    adj = np.ascontiguousarray(np.asarray(adj, dtype=np.int32))
    a1 = np.ascontiguousarray(np.asarray(a1, dtype=np.float32))
    a2 = np.ascontiguousarray(np.asarray(a2, dtype=np.float32))
    beye = np.ascontiguousarray((np.eye(128) * BETA).astype(ml_dtypes.bfloat16))
    eyef = np.ascontiguousarray(np.eye(128, dtype=np.float32))
    in_maps = []
    for c in range(NCORES):
        sl = slice(c * BPC, (c + 1) * BPC)
        in_maps.append(
            {
                "input1": input1[sl],
                "input2": input2[sl],
                "adj": adj[sl],
                "a1": a1,
                "a2": a2,
                "beye": beye,
                "eyef": eyef,
            }
        )
    return in_maps


def _gather(res):
    # device emits out^T (BPC, D, N) bf16; un-transpose + cast (layout only)
    return np.concatenate(
        [
            np.asarray(r["out"]).astype(np.float32).transpose(0, 2, 1)
            for r in res.results
        ],
        axis=0,
    )


def kernel(input1, input2, adj, a1, a2):
    nc = _get_nc()
    res = bass_utils.run_bass_kernel_spmd(
        nc, _make_in_maps(input1, input2, adj, a1, a2),
        core_ids=list(range(NCORES)),
    )
    return _gather(res)


def run_traced(input1, input2, adj, a1, a2, trace_cores=None):
    nc = _get_nc()
    res = bass_utils.run_bass_kernel_spmd(
        nc, _make_in_maps(input1, input2, adj, a1, a2),
        core_ids=list(range(NCORES)),
        trace=True,
        trace_cores=trace_cores or [0],
    )
    return _gather(res), res
